# revision 16
# baseline (speedup 1.0000x reference)
"""Trainium2 Bass kernel for CustomConvWithExtra.

out = conv3x3(x, w_main) + b_main + extra, where extra collapses to a 3x3
border-class table T[b,c,clsh,clsw] (conv of a spatially-constant image).

Design (v7):
 - Data parallel: 1 batch image per NeuronCore (B=8 = 8 cores).
 - fp16 end-to-end on the wire: x is sent as fp16, output is written as fp16
   and upcast to f32 on the host (tolerance is 2e-2; fp16 round-off ~4e-4).
   This halves the dominant HBM write traffic (67MB -> 33.5MB per core).
 - Per output ROW-PAIR: ONE matmul. All 3 kw tap columns are packed into the
   contraction dim: patch rows (kw, d, ci) with d = pair+kh in 0..3, 36 rows
   + 3 static rows (col-0 indicator, col-(W-1) indicator, ones) that fuse
   bias+border terms = 39-row contraction, 128 output partitions
   (pair*64+ch), free dim W=512.  fp16 PE runs 1 row/cycle at 2.4GHz.
 - The kw=1,2 row replicas are read from the SAME HBM planes at +1/+2
   element offsets: plane p holds rows (ch,j) flattened to c*514-blocks, and
   the matmul window for pair j only reads cols [j*514, j*514+512), never the
   last 2 cols of a block, so a flat shifted read gives exactly the
   column-shifted rows (block-crossing garbage lands in unread cols).
 - Output: ob [128, c*512] fp16; HBM layout out[p, ch*8192 + j*512 + x] so
   each chunk's store is ONE DMA with 128 contiguous 16KB descriptors
   (vs 2KB strided lines before -> descriptor overhead dominated).
   Host reassembles [64, 512, 512] and upcasts.
"""

from contextlib import ExitStack

import ml_dtypes
import numpy as np

BF16_NP = ml_dtypes.bfloat16

import concourse.bass as bass
import concourse.tile as tile
from concourse import bacc, mybir
from concourse.bass_utils import run_bass_kernel_spmd

# Problem shapes (hardcoded per contract)
B, CIN, H, W = 8, 3, 512, 512
COUT, E, KS = 64, 3, 3
NCORES = 8
XROW = W + 2       # 514
KP = 39            # patch partitions: 36 = (kw,d,ci) + indL + indR + ones
C = 16             # row-pairs per chunk
PAIRS = H // 2
NCHUNK = PAIRS // C
PLANE = PAIRS * XROW          # 131584 elems per (d,ci) plane
PLANE_PAD = PLANE + 4         # slack so the +2 shifted read of the last plane stays in-bounds
CBLK = C * XROW               # 8224 patch cols per chunk
OBLK = C * W                  # 8192 output cols per chunk
F16 = mybir.dt.float16
BF16 = mybir.dt.bfloat16
F32 = mybir.dt.float32

_cache: dict = {}


def _build():
    nc = bacc.Bacc("TRN2", target_bir_lowering=False, debug=False)
    # xrep[p, ch*CBLK + j*XROW + col] = xp[ci, 2*(ch*C+j)+d, col], p = d*CIN+ci
    xrep = nc.dram_tensor("xrep", [4 * CIN, PLANE_PAD], BF16, kind="ExternalInput").ap()
    wts = nc.dram_tensor("wts", [KP, 3 * 128], BF16, kind="ExternalInput").ap()
    stat = nc.dram_tensor("stat", [3, CBLK], BF16, kind="ExternalInput").ap()
    out = nc.dram_tensor("out", [128, NCHUNK * OBLK], F16, kind="ExternalOutput").ap()

    PBUFS = 4
    with tile.TileContext(nc) as tc, ExitStack() as ctx:
        wpool = ctx.enter_context(tc.tile_pool(name="wpool", bufs=1))
        ppool = ctx.enter_context(tc.tile_pool(name="ppool", bufs=PBUFS))
        opool = ctx.enter_context(tc.tile_pool(name="opool", bufs=3))
        pspool = ctx.enter_context(tc.tile_pool(name="pspool", bufs=8, space="PSUM"))

        # Stationary weights: wtile[k, u*128 + pair*64 + co], u = row-class
        wtile = wpool.tile([KP, 3 * 128], BF16)
        nc.sync.dma_start(wtile[:, :], wts[:, :])

        # Patch buffers; static rows 36:39 loaded once per physical buffer.
        patch_tiles = []
        for s in range(PBUFS):
            pt = ppool.tile([KP, CBLK], BF16, name=f"patch{s}", tag="patch")
            nc.sync.dma_start(pt[36:39, :], stat[:, :])
            patch_tiles.append(pt)

        for ch in range(NCHUNK):
            pt = patch_tiles[ch % PBUFS]
            # Input loads: rows (kw,d,ci) <- plane (d,ci) shifted by kw elems.
            # Batched two chunks at a time so the 6 read descriptors per queue
            # arrive contiguously (fewer HBM read/write turnarounds).
            if ch % 2 == 0:
                for cc in (ch, ch + 1):
                    ptc = patch_tiles[cc % PBUFS]
                    for kw in range(3):
                        src = bass.AP(
                            xrep.tensor,
                            cc * CBLK + kw,
                            [[PLANE_PAD, 4 * CIN], [1, CBLK]],
                        )
                        nc.gpsimd.dma_start(ptc[12 * kw : 12 * kw + 12, :], src)

            if ch % 2 == 0:
                ob = opool.tile([128, 2 * OBLK], F16, name="ob", tag="ob")
            ob_off = (ch % 2) * OBLK
            for j in range(C):
                pairidx = ch * C + j
                u = 0 if pairidx == 0 else (2 if pairidx == PAIRS - 1 else 1)
                ps = pspool.tile([128, W], F32, name="ps", tag="ps")
                nc.tensor.matmul(
                    ps[:, :],
                    wtile[:, u * 128 : (u + 1) * 128],
                    pt[:, j * XROW : j * XROW + W],
                    start=True,
                    stop=True,
                )
                # scalar owns odd j (incl. j=C-1) and issues the store right
                # after its own closing copy, so the store wait is locally
                # satisfied; vector owns even j.  16KB descriptors: splitting
                # them in half measurably lowers per-queue throughput.
                if j % 2 == 1:
                    nc.scalar.copy(ob[:, ob_off + j * W : ob_off + (j + 1) * W], ps[:, :])
                else:
                    nc.vector.tensor_copy(ob[:, ob_off + j * W : ob_off + (j + 1) * W], ps[:, :])

            # One store per chunk PAIR: 128 x 32KB descriptors (32KB writes
            # run at full per-queue rate; halves read/write turnarounds).
            if ch % 2 == 1:
                dst = bass.AP(
                    out.tensor,
                    (ch - 1) * OBLK,
                    [[NCHUNK * OBLK, 128], [1, 2 * OBLK]],
                )
                nc.scalar.dma_start(dst, ob[:, :])

    nc.compile()
    return nc


def _host_prep(x, v, wm, bm, we, be):
    """Per-core inputs: shifted row-planes (fp16), fused weights, statics."""
    Bb = x.shape[0]
    vr = v.reshape(Bb, COUT, E).astype(np.float64)

    # Border-class table for the 'extra' convs of a constant image.
    sets = {0: [1, 2], 1: [0, 1, 2], 2: [0, 1]}
    Mcl = np.zeros((COUT, E, 3, 3), np.float64)
    we64 = we.astype(np.float64)
    for ch_ in range(3):
        for cw in range(3):
            Mcl[:, :, ch_, cw] = we64[:, :, sets[ch_], :][:, :, :, sets[cw]].sum((2, 3))
    T = (
        np.einsum("bce,cehw->bchw", vr, Mcl)
        + bm.astype(np.float64)[None, :, None, None]
        + be.astype(np.float64)[None, :, None, None]
    )

    # xrep[b, d*CIN+ci, (ch*C+j)*XROW + col] = xp[b, ci, 2*(ch*C+j)+d, col]
    xp = np.zeros((Bb, CIN, H + 2, XROW), BF16_NP)
    xp[:, :, 1 : H + 1, 1 : W + 1] = x.astype(BF16_NP)
    xrep = np.zeros((Bb, 4 * CIN, PLANE_PAD), BF16_NP)
    view = xrep[:, :, :PLANE].reshape(Bb, 4, CIN, PAIRS, XROW)
    for d in range(4):
        view[:, d] = xp[:, :, d : d + H : 2, :]

    # Fused weights: row q = kw*12 + d*CIN + ci, col u*128 + pair*64 + co
    pair_cls = {0: (0, 1), 1: (1, 1), 2: (1, 2)}
    wts = np.zeros((Bb, KP, 3, 128), np.float64)
    for u in range(3):
        for kw in range(KS):
            for pair in range(2):
                cols = slice(pair * 64, pair * 64 + 64)
                for d in range(4):
                    kh = d - pair
                    if 0 <= kh < KS:
                        for ci in range(CIN):
                            wts[:, kw * 12 + d * CIN + ci, u, cols] = wm[:, ci, kh, kw]
                cls = pair_cls[u][pair]
                wts[:, 36, u, cols] = T[:, :, cls, 0] - T[:, :, cls, 1]
                wts[:, 37, u, cols] = T[:, :, cls, 2] - T[:, :, cls, 1]
                wts[:, 38, u, cols] = T[:, :, cls, 1]
    wts = wts.reshape(Bb, KP, 3 * 128).astype(BF16_NP)

    stat = np.zeros((3, CBLK), BF16_NP)
    stat[0, 0::XROW] = 1.0          # rhs col 0 of each window
    stat[1, W - 1 :: XROW] = 1.0    # rhs col W-1 of each window
    stat[2, :] = 1.0                # ones row (bias + interior border term)
    return xrep, wts, stat


def kernel(**inputs) -> np.ndarray:
    x = np.ascontiguousarray(np.asarray(inputs["x"], np.float32))
    v = np.asarray(inputs["extra_inputs"], np.float32)
    wm = np.asarray(inputs["w_main"], np.float32)
    bm = np.asarray(inputs["b_main"], np.float32)
    we = np.asarray(inputs["w_extra"], np.float32)
    be = np.asarray(inputs["b_extra"], np.float32)

    xrep, wts, stat = _host_prep(x, v, wm, bm, we, be)

    if "nc" not in _cache:
        _cache["nc"] = _build()
    nc = _cache["nc"]

    in_maps = [{"xrep": xrep[b], "wts": wts[b], "stat": stat} for b in range(B)]
    res = run_bass_kernel_spmd(nc, in_maps, list(range(NCORES)))
    outs = []
    for b in range(B):
        ob = res.results[b]["out"]  # [128, NCHUNK*OBLK] fp16
        ob = ob.reshape(2, 64, NCHUNK, C, W).transpose(1, 2, 3, 0, 4)
        outs.append(ob.reshape(COUT, H, W).astype(np.float32))
    return np.stack(outs)
